# revision 26
# baseline (speedup 1.0000x reference)
"""TRN2 Bass kernel for per-sample low-rank adapter routing (moe_routing).

Computation (per batch b):
    gate  = softmax(MLP(LN(ctr[b])))              # tiny, done on host (f32)
    A     = (gate @ Wa.T).reshape(R, D_IN)        # [8, 2048]   host
    B     = (gate @ Wb.T).reshape(R, D_OUT)*scale # [8, 2048]   host
    out_b = (x_b @ A.T) @ B                       # [2048, 2048]  <- device

Device side is memory-bound: reads x (16 MiB/core), writes out (16 MiB/core).
Sharding: batch dim (8) across the 8 NeuronCores, adapters replicated.

Key design choices (measured on HW, see trace iterations):
 * Host ships x TRANSPOSED and macro-tiled ([m, p, c, s]) so the contraction
   dim lands on SBUF partitions straight from DMA -- no on-chip transposes,
   and every dma_start moves >=4KB-contiguous runs per partition (~25 GB/s
   per DMA engine, the practical cap).
 * fp16 for x / A^T / B and for the output (host upcasts): halves DMA bytes
   (the kernel is DMA-bound) at ~4.7e-4 relative error -- same error class
   as the PE's reduced-precision f32r mode, far better than bf16.
 * mm1 (xa^T = A x^T) packs its M=8 matmuls 4x into PE column-tiles
   (tile_position=(0,32g)); a K=1 zero-matmul pre-clears PSUM so the
   never-written partitions read 0.0 and mm2 contracts K=104 against a
   zero-padded B (bm4).
 * Software-pipelined emission: macro m's mm1 quarter-groups interleave
   with macro m-1's mm2 t-blocks in the in-order PE stream.
 * DMA trigger streams are segregated: loads on Sync (HWDGE), PSUM
   evacuation split across ScalarE+VectorE, stores on GpSimd (SWDGE) so
   store triggers (which block on evac) never stall load dispatch.

Measured: ~62 us HW exec per core (f32 I/O roofline would be ~90 us;
fp16-I/O DMA floor is ~45 us + ~12 us fixed NEFF pre/postamble).
"""
import sys

sys.path.insert(0, '/opt/trn_rl_repo')

import numpy as np

import concourse.bacc as bacc
import concourse.mybir as mybir
import concourse.tile as tile
from concourse.bass_utils import run_bass_kernel_spmd

R = 8
D_IN = 2048
D_OUT = 2048
SEQ = 2048
BS = 8
SCALING = 16.0 / R
LN_EPS = 1e-5
TEMPERATURE = 1.0

F32 = mybir.dt.float32
F32R = mybir.dt.float32r
F16 = mybir.dt.float16

MACRO = 512                      # seq rows per macro tile
N_MACRO = SEQ // MACRO           # 4
N_SUB = MACRO // 128             # 4 row sub-blocks per macro
N_KC = D_IN // 128               # 16 contraction chunks

_COMPILED = None


def _build_program():
    nc = bacc.Bacc("TRN2", target_bir_lowering=False, debug=False, num_devices=8)
    # host pre-tiles x^T macro-major [m, p, c, s]: each quarter-macro load is
    # one dma_start with 4KB-contiguous runs per partition.
    xt_d = nc.dram_tensor(
        "xt", [N_MACRO, 128, N_KC, MACRO], F16, kind="ExternalInput").ap()
    # host pre-permutes A^T to partition-major [128, N_KC, R]
    at_d = nc.dram_tensor("at", [128, N_KC, R], F16, kind="ExternalInput").ap()
    # bm4: B rows replicated into 4 column-tile groups at partition 32g+r,
    # zero rows elsewhere (multiplied against psum garbage partitions)
    bm_d = nc.dram_tensor("bm4", [128, D_OUT], F16, kind="ExternalInput").ap()
    z_d = nc.dram_tensor("z", [1, 512], F16, kind="ExternalInput").ap()
    out_d = nc.dram_tensor("out", [SEQ, D_OUT], F16, kind="ExternalOutput").ap()

    with tile.TileContext(nc) as tc:
        with tc.tile_pool(name="const", bufs=1) as cpool, \
             tc.tile_pool(name="xtp", bufs=12) as xtp, \
             tc.tile_pool(name="evac", bufs=4) as evac, \
             tc.tile_pool(name="ps", bufs=3, space="PSUM") as ps, \
             tc.tile_pool(name="ps2", bufs=2, space="PSUM") as ps2:
            at_r = cpool.tile([128, N_KC, R], F16, tag="at_r")
            bm_r = cpool.tile([128, D_OUT], F16, tag="bm_r")
            z_r = cpool.tile([1, 512], F16, tag="z_r")

            NQ = 4               # kc chunks per load quarter
            NGRP = N_KC // NQ    # 4 quarter groups per macro

            def emit_loads(m, xt_qs):
                qs = []
                for q in range(NGRP):
                    t_ = xtp.tile([128, NQ, MACRO], F16, tag="xt_q")
                    nc.sync.dma_start(t_[:], xt_d[m, :, q * NQ:(q + 1) * NQ, :])
                    qs.append(t_)
                xt_qs[m] = qs

            def emit_mm1_group(m, q, xa_ps_m, xt_qs):
                if q == 0:
                    # K=1 zero matmul clears the psum tile (sets has_written
                    # so packed accumulations land on zeros; never-written
                    # partitions read back 0.0, not stale garbage)
                    nc.tensor.matmul(
                        xa_ps_m[:], z_r[:, 0:128], z_r[:, 0:MACRO],
                        start=True, stop=False, skip_group_check=True,
                    )
                for i in range(NQ):
                    kc = q * NQ + i
                    g = kc % 4
                    nc.tensor.matmul(
                        xa_ps_m[32 * g:32 * g + R, :],
                        at_r[:, kc, :],
                        xt_qs[m][q][:, i, :],
                        start=False, stop=(kc == N_KC - 1),
                        tile_position=(0, 32 * g),
                        skip_group_check=True,
                    )

            def emit_mm2_block(m, t, xa_r_m):
                o_sb = evac.tile([128, D_OUT], F16, tag="o_sb")
                for half in range(2):
                    o_ps = ps.tile([128, 1024], F32, tag="o_ps")
                    for j in range(2):
                        nc.tensor.matmul(
                            o_ps[:, j * 512:(j + 1) * 512],
                            xa_r_m[0:104, t * 128:(t + 1) * 128],
                            bm_r[0:104, half * 1024 + j * 512:
                                 half * 1024 + (j + 1) * 512],
                            start=True, stop=True,
                        )
                    # split psum evacuation across the two free engines
                    if half == 0:
                        nc.scalar.copy(o_sb[:, 0:1024], o_ps[:])
                    else:
                        nc.vector.tensor_copy(o_sb[:, 1024:2048], o_ps[:])
                # stores on SWDGE (idle GpSimd): their data-waits must not
                # stall the Sync (loads) or Scalar (evac) streams
                nc.gpsimd.dma_start(
                    out_d[m * MACRO + t * 128:m * MACRO + (t + 1) * 128, :],
                    o_sb[:],
                )

            # software pipeline: macro m's mm1 quarter-groups interleave with
            # macro m-1's mm2 t-blocks so the in-order PE queue always has
            # ready work while quarter loads are in flight
            xt_qs = {}
            xa_r_prev = None
            emit_loads(0, xt_qs)
            # tiny const loads go after the first x quarters on the queue
            nc.sync.dma_start(at_r[:], at_d[:])
            nc.sync.dma_start(bm_r[:], bm_d[:])
            nc.sync.dma_start(z_r[:], z_d[:])
            for m in range(N_MACRO):
                if m + 1 < N_MACRO:
                    emit_loads(m + 1, xt_qs)
                xa_ps_m = ps2.tile([128, MACRO], F32, tag="xa_ps")
                for q in range(NGRP):
                    if xa_r_prev is not None:
                        emit_mm2_block(m - 1, q, xa_r_prev)
                    emit_mm1_group(m, q, xa_ps_m, xt_qs)
                xa_r_m = evac.tile([128, MACRO], F16, tag="xa_r")
                # split the evac per t-slice so each mm2 block of this macro
                # starts as soon as its own 128-col slice is in SBUF
                for t4 in range(N_SUB):
                    eng = nc.vector.tensor_copy if t4 % 2 == 0 else nc.scalar.copy
                    eng(xa_r_m[0:104, t4 * 128:(t4 + 1) * 128],
                        xa_ps_m[0:104, t4 * 128:(t4 + 1) * 128])
                xa_r_prev = xa_r_m
                del xt_qs[m]
            for t in range(N_SUB):
                # tail blocks store per half so the last exposed DMA is 0.25MB
                o_sb = evac.tile([128, D_OUT], F16, tag="o_sb")
                for half in range(2):
                    o_ps = ps.tile([128, 1024], F32, tag="o_ps")
                    for j in range(2):
                        nc.tensor.matmul(
                            o_ps[:, j * 512:(j + 1) * 512],
                            xa_r_prev[0:104, t * 128:(t + 1) * 128],
                            bm_r[0:104, half * 1024 + j * 512:
                                 half * 1024 + (j + 1) * 512],
                            start=True, stop=True,
                        )
                    if half == 0:
                        nc.scalar.copy(o_sb[:, 0:1024], o_ps[:])
                    else:
                        nc.vector.tensor_copy(o_sb[:, 1024:2048], o_ps[:])
                    nc.gpsimd.dma_start(
                        out_d[(N_MACRO - 1) * MACRO + t * 128:
                              (N_MACRO - 1) * MACRO + (t + 1) * 128,
                              half * 1024:(half + 1) * 1024],
                        o_sb[:, half * 1024:(half + 1) * 1024],
                    )
    nc.compile()
    return nc


def _gating_host(ctr, ln_gamma, ln_beta, W1, b1, W2, b2):
    """Replicates the reference gating MLP in numpy float32. ctr: [bs, 32]."""
    ctr = ctr.astype(np.float32)
    mu = np.mean(ctr, axis=-1, keepdims=True, dtype=np.float32)
    d = ctr - mu
    var = np.mean(np.square(d), axis=-1, keepdims=True, dtype=np.float32)
    z = d * (1.0 / np.sqrt(var + np.float32(LN_EPS))) * ln_gamma + ln_beta
    h = np.maximum(z @ W1.T + b1, np.float32(0.0))
    g = h @ W2.T + b2
    g = g / np.float32(TEMPERATURE)
    g = g - np.max(g, axis=-1, keepdims=True)
    e = np.exp(g)
    return (e / np.sum(e, axis=-1, keepdims=True)).astype(np.float32)


def kernel(x, ctr_hidden_states, ln_gamma, ln_beta, W1, b1, W2, b2, Wa, Wb):
    global _COMPILED
    x = np.asarray(x, dtype=np.float32)
    ctr = np.asarray(ctr_hidden_states, dtype=np.float32)
    ln_gamma = np.asarray(ln_gamma, dtype=np.float32)
    ln_beta = np.asarray(ln_beta, dtype=np.float32)
    W1 = np.asarray(W1, dtype=np.float32)
    b1 = np.asarray(b1, dtype=np.float32)
    W2 = np.asarray(W2, dtype=np.float32)
    b2 = np.asarray(b2, dtype=np.float32)
    Wa = np.asarray(Wa, dtype=np.float32)
    Wb = np.asarray(Wb, dtype=np.float32)

    gate = _gating_host(ctr, ln_gamma, ln_beta, W1, b1, W2, b2)   # [bs, 4]
    A = (gate @ Wa.T).reshape(BS, R, D_IN)                         # [bs, 8, 2048]
    Bm = (gate @ Wb.T).reshape(BS, R, D_OUT) * np.float32(SCALING)

    if _COMPILED is None:
        _COMPILED = _build_program()
    nc = _COMPILED

    in_maps = []
    for b in range(BS):
        # at: A^T [2048, 8] -> partition-major [128, N_KC, R]
        at_pm = np.ascontiguousarray(
            A[b].T.reshape(N_KC, 128, R).transpose(1, 0, 2)).astype(np.float16)
        # x^T [d, s] -> macro-tiled [m, p(128 of d), c(16 d-chunks), s(512)]
        xt_pm = np.ascontiguousarray(
            x[b].T.reshape(N_KC, 128, N_MACRO, MACRO).transpose(2, 1, 0, 3)
        ).astype(np.float16)
        bm4 = np.zeros((128, D_OUT), dtype=np.float16)
        for g in range(4):
            bm4[32 * g:32 * g + R, :] = Bm[b].astype(np.float16)
        in_maps.append({
            "xt": xt_pm,
            "at": at_pm,
            "bm4": bm4,
            "z": np.zeros((1, 512), dtype=np.float16),
        })
    core_ids = list(range(BS))
    res = run_bass_kernel_spmd(nc, in_maps, core_ids)
    out = np.stack([res.results[b]["out"] for b in range(BS)], axis=0)
    return out.astype(np.float32)


# revision 27
# speedup vs baseline: 1.0037x; 1.0037x over previous
"""TRN2 Bass kernel for per-sample low-rank adapter routing (moe_routing).

Computation (per batch b):
    gate  = softmax(MLP(LN(ctr[b])))              # tiny, done on host (f32)
    A     = (gate @ Wa.T).reshape(R, D_IN)        # [8, 2048]   host
    B     = (gate @ Wb.T).reshape(R, D_OUT)*scale # [8, 2048]   host
    out_b = (x_b @ A.T) @ B                       # [2048, 2048]  <- device

Device side is memory-bound: reads x (16 MiB/core), writes out (16 MiB/core).
Sharding: batch dim (8) across the 8 NeuronCores, adapters replicated.

Key design choices (measured on HW, see trace iterations):
 * Host ships x TRANSPOSED and macro-tiled ([m, p, c, s]) so the contraction
   dim lands on SBUF partitions straight from DMA -- no on-chip transposes,
   and every dma_start moves >=4KB-contiguous runs per partition (~25 GB/s
   per DMA engine, the practical cap).
 * fp16 for x / A^T / B and for the output (host upcasts): halves DMA bytes
   (the kernel is DMA-bound) at ~4.7e-4 relative error -- same error class
   as the PE's reduced-precision f32r mode, far better than bf16.
 * mm1 (xa^T = A x^T) packs its M=8 matmuls 4x into PE column-tiles
   (tile_position=(0,32g)); a K=1 zero-matmul pre-clears PSUM so the
   never-written partitions read 0.0 and mm2 contracts K=104 against a
   zero-padded B (bm4).
 * Software-pipelined emission: macro m's mm1 quarter-groups interleave
   with macro m-1's mm2 t-blocks in the in-order PE stream.
 * DMA trigger streams are segregated: loads on Sync (HWDGE), PSUM
   evacuation split across ScalarE+VectorE, stores on GpSimd (SWDGE) so
   store triggers (which block on evac) never stall load dispatch.

Measured: ~62 us HW exec per core (f32 I/O roofline would be ~90 us;
fp16-I/O DMA floor is ~45 us + ~12 us fixed NEFF pre/postamble).
"""
import sys

sys.path.insert(0, '/opt/trn_rl_repo')

import numpy as np

import concourse.bacc as bacc
import concourse.mybir as mybir
import concourse.tile as tile
from concourse.bass_utils import run_bass_kernel_spmd

R = 8
D_IN = 2048
D_OUT = 2048
SEQ = 2048
BS = 8
SCALING = 16.0 / R
LN_EPS = 1e-5
TEMPERATURE = 1.0

F32 = mybir.dt.float32
F32R = mybir.dt.float32r
F16 = mybir.dt.float16

MACRO = 512                      # seq rows per macro tile
N_MACRO = SEQ // MACRO           # 4
N_SUB = MACRO // 128             # 4 row sub-blocks per macro
N_KC = D_IN // 128               # 16 contraction chunks

_COMPILED = None


def _build_program():
    nc = bacc.Bacc("TRN2", target_bir_lowering=False, debug=False, num_devices=8)
    # host pre-tiles x^T macro-major [m, p, c, s]: each quarter-macro load is
    # one dma_start with 4KB-contiguous runs per partition.
    xt_d = nc.dram_tensor(
        "xt", [N_MACRO, 128, N_KC, MACRO], F16, kind="ExternalInput").ap()
    # host pre-permutes A^T to partition-major [128, N_KC, R]
    at_d = nc.dram_tensor("at", [128, N_KC, R], F16, kind="ExternalInput").ap()
    # bm4: B rows replicated into 4 column-tile groups at partition 32g+r,
    # zero rows elsewhere (multiplied against psum garbage partitions)
    bm_d = nc.dram_tensor("bm4", [128, D_OUT], F16, kind="ExternalInput").ap()
    z_d = nc.dram_tensor("z", [1, 512], F16, kind="ExternalInput").ap()
    out_d = nc.dram_tensor("out", [SEQ, D_OUT], F16, kind="ExternalOutput").ap()

    with tile.TileContext(nc) as tc:
        with tc.tile_pool(name="const", bufs=1) as cpool, \
             tc.tile_pool(name="xtp", bufs=12) as xtp, \
             tc.tile_pool(name="evac", bufs=4) as evac, \
             tc.tile_pool(name="ps", bufs=3, space="PSUM") as ps, \
             tc.tile_pool(name="ps2", bufs=2, space="PSUM") as ps2:
            at_r = cpool.tile([128, N_KC, R], F16, tag="at_r")
            bm_r = cpool.tile([128, D_OUT], F16, tag="bm_r")
            z_r = cpool.tile([1, 512], F16, tag="z_r")

            NQ = 4               # kc chunks per load quarter
            NGRP = N_KC // NQ    # 4 quarter groups per macro

            def emit_loads(m, xt_qs):
                qs = []
                for q in range(NGRP):
                    t_ = xtp.tile([128, NQ, MACRO], F16, tag="xt_q")
                    nc.sync.dma_start(t_[:], xt_d[m, :, q * NQ:(q + 1) * NQ, :])
                    qs.append(t_)
                xt_qs[m] = qs

            def emit_mm1_group(m, q, xa_ps_m, xt_qs):
                if q == 0:
                    # K=1 zero matmul clears the psum tile (sets has_written
                    # so packed accumulations land on zeros; never-written
                    # partitions read back 0.0, not stale garbage)
                    nc.tensor.matmul(
                        xa_ps_m[:], z_r[:, 0:128], z_r[:, 0:MACRO],
                        start=True, stop=False, skip_group_check=True,
                    )
                for i in range(NQ):
                    kc = q * NQ + i
                    g = kc % 4
                    nc.tensor.matmul(
                        xa_ps_m[32 * g:32 * g + R, :],
                        at_r[:, kc, :],
                        xt_qs[m][q][:, i, :],
                        start=False, stop=(kc == N_KC - 1),
                        tile_position=(0, 32 * g),
                        skip_group_check=True,
                    )

            def emit_mm2_block(m, t, xa_r_m):
                o_sb = evac.tile([128, D_OUT], F16, tag="o_sb")
                for half in range(2):
                    o_ps = ps.tile([128, 1024], F32, tag="o_ps")
                    for j in range(2):
                        nc.tensor.matmul(
                            o_ps[:, j * 512:(j + 1) * 512],
                            xa_r_m[0:104, t * 128:(t + 1) * 128],
                            bm_r[0:104, half * 1024 + j * 512:
                                 half * 1024 + (j + 1) * 512],
                            start=True, stop=True,
                        )
                    # split psum evacuation across the two free engines
                    if half == 0:
                        nc.scalar.copy(o_sb[:, 0:1024], o_ps[:])
                    else:
                        nc.vector.tensor_copy(o_sb[:, 1024:2048], o_ps[:])
                # stores on SWDGE (idle GpSimd): their data-waits must not
                # stall the Sync (loads) or Scalar (evac) streams
                nc.gpsimd.dma_start(
                    out_d[m * MACRO + t * 128:m * MACRO + (t + 1) * 128, :],
                    o_sb[:],
                )

            # software pipeline: macro m's mm1 quarter-groups interleave with
            # macro m-1's mm2 t-blocks so the in-order PE queue always has
            # ready work while quarter loads are in flight
            xt_qs = {}
            xa_r_prev = None
            emit_loads(0, xt_qs)
            # tiny const loads go after the first x quarters on the queue
            nc.sync.dma_start(at_r[:], at_d[:])
            nc.sync.dma_start(bm_r[:], bm_d[:])
            nc.sync.dma_start(z_r[:], z_d[:])
            for m in range(N_MACRO):
                if m + 1 < N_MACRO:
                    emit_loads(m + 1, xt_qs)
                xa_ps_m = ps2.tile([128, MACRO], F32, tag="xa_ps")
                for q in range(NGRP):
                    if xa_r_prev is not None:
                        emit_mm2_block(m - 1, q, xa_r_prev)
                    emit_mm1_group(m, q, xa_ps_m, xt_qs)
                xa_r_m = evac.tile([128, MACRO], F16, tag="xa_r")
                # split the evac per t-slice so each mm2 block of this macro
                # starts as soon as its own 128-col slice is in SBUF
                for t4 in range(N_SUB):
                    eng = nc.vector.tensor_copy if t4 % 2 == 0 else nc.scalar.copy
                    eng(xa_r_m[0:104, t4 * 128:(t4 + 1) * 128],
                        xa_ps_m[0:104, t4 * 128:(t4 + 1) * 128])
                xa_r_prev = xa_r_m
                del xt_qs[m]
            for t in range(N_SUB):
                # tail blocks store per half so the last exposed DMA is 0.25MB
                o_sb = evac.tile([128, D_OUT], F16, tag="o_sb")
                for half in range(2):
                    o_ps = ps.tile([128, 1024], F32, tag="o_ps")
                    for j in range(2):
                        nc.tensor.matmul(
                            o_ps[:, j * 512:(j + 1) * 512],
                            xa_r_prev[0:104, t * 128:(t + 1) * 128],
                            bm_r[0:104, half * 1024 + j * 512:
                                 half * 1024 + (j + 1) * 512],
                            start=True, stop=True,
                        )
                    if half == 0:
                        nc.scalar.copy(o_sb[:, 0:1024], o_ps[:])
                    else:
                        nc.vector.tensor_copy(o_sb[:, 1024:2048], o_ps[:])
                    # tail stores ride the HWDGE sync queue (loads are long
                    # done) -- avoids the slow SWDGE drain in the exit barrier
                    nc.sync.dma_start(
                        out_d[(N_MACRO - 1) * MACRO + t * 128:
                              (N_MACRO - 1) * MACRO + (t + 1) * 128,
                              half * 1024:(half + 1) * 1024],
                        o_sb[:, half * 1024:(half + 1) * 1024],
                    )
    nc.compile()
    return nc


def _gating_host(ctr, ln_gamma, ln_beta, W1, b1, W2, b2):
    """Replicates the reference gating MLP in numpy float32. ctr: [bs, 32]."""
    ctr = ctr.astype(np.float32)
    mu = np.mean(ctr, axis=-1, keepdims=True, dtype=np.float32)
    d = ctr - mu
    var = np.mean(np.square(d), axis=-1, keepdims=True, dtype=np.float32)
    z = d * (1.0 / np.sqrt(var + np.float32(LN_EPS))) * ln_gamma + ln_beta
    h = np.maximum(z @ W1.T + b1, np.float32(0.0))
    g = h @ W2.T + b2
    g = g / np.float32(TEMPERATURE)
    g = g - np.max(g, axis=-1, keepdims=True)
    e = np.exp(g)
    return (e / np.sum(e, axis=-1, keepdims=True)).astype(np.float32)


def kernel(x, ctr_hidden_states, ln_gamma, ln_beta, W1, b1, W2, b2, Wa, Wb):
    global _COMPILED
    x = np.asarray(x, dtype=np.float32)
    ctr = np.asarray(ctr_hidden_states, dtype=np.float32)
    ln_gamma = np.asarray(ln_gamma, dtype=np.float32)
    ln_beta = np.asarray(ln_beta, dtype=np.float32)
    W1 = np.asarray(W1, dtype=np.float32)
    b1 = np.asarray(b1, dtype=np.float32)
    W2 = np.asarray(W2, dtype=np.float32)
    b2 = np.asarray(b2, dtype=np.float32)
    Wa = np.asarray(Wa, dtype=np.float32)
    Wb = np.asarray(Wb, dtype=np.float32)

    gate = _gating_host(ctr, ln_gamma, ln_beta, W1, b1, W2, b2)   # [bs, 4]
    A = (gate @ Wa.T).reshape(BS, R, D_IN)                         # [bs, 8, 2048]
    Bm = (gate @ Wb.T).reshape(BS, R, D_OUT) * np.float32(SCALING)

    if _COMPILED is None:
        _COMPILED = _build_program()
    nc = _COMPILED

    in_maps = []
    for b in range(BS):
        # at: A^T [2048, 8] -> partition-major [128, N_KC, R]
        at_pm = np.ascontiguousarray(
            A[b].T.reshape(N_KC, 128, R).transpose(1, 0, 2)).astype(np.float16)
        # x^T [d, s] -> macro-tiled [m, p(128 of d), c(16 d-chunks), s(512)]
        xt_pm = np.ascontiguousarray(
            x[b].T.reshape(N_KC, 128, N_MACRO, MACRO).transpose(2, 1, 0, 3)
        ).astype(np.float16)
        bm4 = np.zeros((128, D_OUT), dtype=np.float16)
        for g in range(4):
            bm4[32 * g:32 * g + R, :] = Bm[b].astype(np.float16)
        in_maps.append({
            "xt": xt_pm,
            "at": at_pm,
            "bm4": bm4,
            "z": np.zeros((1, 512), dtype=np.float16),
        })
    core_ids = list(range(BS))
    res = run_bass_kernel_spmd(nc, in_maps, core_ids)
    out = np.stack([res.results[b]["out"] for b in range(BS)], axis=0)
    return out.astype(np.float32)
